# revision 13
# baseline (speedup 1.0000x reference)
"""Bass/Trainium2 kernel for nn_BiDirectionalCrossAttentionLayer.

Sharding: 8 cores = batch(4) x head-group(2). Each core computes, for its
batch b and its 4 heads, the full 4-stream cross-attention + the 256 output
rows (t = hg*256 .. hg*256+255) of every stream. The reference's
"transpose(1,2) ... transpose/reshape" scramble maps output row t to
(head t//64, head-dim t%64) over all sequence positions, so a head-split of
attention is exactly an output-row split of everything after it.

All matmuls in bf16 (fp32 accumulate); residuals/LN in fp32.
"""

import os
import numpy as np
import ml_dtypes

import concourse.bacc as bacc
import concourse.bass as bass
import concourse.tile as tile
from concourse import mybir
from concourse.bass_utils import run_bass_kernel_spmd
from concourse.masks import make_identity

BF16 = ml_dtypes.bfloat16
F32 = np.float32

NS, B, S, E, H, HD = 4, 4, 512, 512, 8, 64
SCALE = HD ** -0.5
LN_EPS = 1e-5
P = 128
HG = 2            # head groups == cores per batch
HPC = H // HG // 2  # head-pairs per core = 2
HC = H // HG      # heads per core = 4
TG = S // HG      # output rows per core per stream = 256
TS = TG // P      # row tiles per core = 2
ET = E // P       # embedding tiles = 4
KT = S // P       # key/seq tiles = 4
FT = 4 * E // P   # ffn hidden tiles = 16
N_CORES = B * HG

AF = mybir.ActivationFunctionType
ALU = mybir.AluOpType
AX = mybir.AxisListType
DT_BF = mybir.dt.bfloat16
DT_F32 = mybir.dt.float32


def _build_program():
    nc = bacc.Bacc("TRN2", target_bir_lowering=False, debug=False)

    def din(name, shape, dt=DT_BF):
        return nc.dram_tensor(name, list(shape), dt, kind="ExternalInput").ap()

    xT_d = din("xT", (NS, P, ET, S))            # xT[n,p,et,s] = x[n,b,s,et*128+p]
    x32_d = din("x32", (NS, P, TS, E), DT_F32)  # x rows t-slice
    wq_d = din("wq", (NS, P, ET, HC * HD))      # Wq[n, e, hg*256 + c]
    wk_d = din("wk", (NS, P, ET, HC * HD))
    wv_d = din("wv", (NS, P, ET, HC * HD))
    wo_d = din("wo", (NS, P, ET, E))            # Wo[n]/NS, rows e
    w1_d = din("w1", (NS, P, ET, 4 * E))
    w2_d = din("w2", (NS, P, FT, E))
    cmat_d = din("cmat", (P, NS * NS), DT_F32)  # SCALE*inter broadcast on p
    bo_d = din("bo", (NS, E), DT_F32)
    g1_d = din("g1", (NS, E), DT_F32)
    b1_d = din("b1", (NS, E), DT_F32)
    g2_d = din("g2", (NS, E), DT_F32)
    b2_d = din("b2", (NS, E), DT_F32)
    bf1_d = din("bf1", (NS, P, FT), DT_F32)     # bf1[n, fs*128+p]
    bf2_d = din("bf2", (NS, E), DT_F32)
    out_d = nc.dram_tensor("out", [NS, P, TS, E], DT_F32, kind="ExternalOutput").ap()

    with tile.TileContext(nc) as tc:
        with tc.tile_pool(name="const", bufs=1) as const:
            ident = const.tile([P, P], DT_BF)
            make_identity(nc, ident[:])
            cmat_sb = const.tile([P, NS * NS], DT_F32)
            nc.sync.dma_start(cmat_sb[:], cmat_d[:])
            eps_sb = const.tile([P, 1], DT_F32)
            nc.gpsimd.memset(eps_sb[:], LN_EPS)

            # long-lived activations
            r1 = const.tile([P, NS, TS, E], DT_F32)
            r1T = const.tile([P, NS, ET, TG], DT_BF)

            scopeB = tc.alloc_tile_pool(name="scopeB", bufs=1)
            x32 = scopeB.tile([P, NS, TS, E], DT_F32)
            att = scopeB.tile([P, NS, KT, HC * HD], DT_F32)  # att_std accum
            attb = scopeB.tile([P, NS, KT, HC * HD], DT_BF)
            nc.gpsimd.memset(att[:], 0.0)
            for n in range(NS):
                nc.sync.dma_start(x32[:, n], x32_d[n])

            scopeA = tc.alloc_tile_pool(name="scopeA", bufs=1)
            qT = scopeA.tile([P, NS, HPC, S], DT_BF)   # [d-pair rows, n, hp, q]
            kT = scopeA.tile([P, NS, HPC, S], DT_BF)
            vex = scopeA.tile([P, NS, KT, HC, HD + 1], DT_BF)
            nc.gpsimd.memset(vex[:, :, :, :, HD:HD + 1], 1.0)

            # ---------------- Phase 1: QKV projections ----------------
            with tc.tile_pool(name="p1w", bufs=1) as p1w, \
                 tc.tile_pool(name="p1ps", bufs=2, space="PSUM") as p1ps:
                xTs = p1w.tile([P, NS, ET, S], DT_BF)
                wqs = p1w.tile([P, NS, ET, HC * HD], DT_BF)
                wks = p1w.tile([P, NS, ET, HC * HD], DT_BF)
                wvs = p1w.tile([P, NS, ET, HC * HD], DT_BF)
                for n in range(NS):
                    nc.sync.dma_start(xTs[:, n], xT_d[n])
                    nc.sync.dma_start(wqs[:, n], wq_d[n])
                    nc.sync.dma_start(wks[:, n], wk_d[n])
                    nc.sync.dma_start(wvs[:, n], wv_d[n])

                for n in range(NS):
                    for hp in range(HPC):
                        ps_q = p1ps.tile([P, S], DT_F32, tag="psq")
                        ps_k = p1ps.tile([P, S], DT_F32, tag="psk")
                        for et in range(ET):
                            nc.tensor.matmul(
                                ps_q[:], wqs[:, n, et, hp * P:(hp + 1) * P],
                                xTs[:, n, et], start=(et == 0), stop=(et == ET - 1))
                        nc.vector.tensor_copy(qT[:, n, hp], ps_q[:])
                        for et in range(ET):
                            nc.tensor.matmul(
                                ps_k[:], wks[:, n, et, hp * P:(hp + 1) * P],
                                xTs[:, n, et], start=(et == 0), stop=(et == ET - 1))
                        nc.vector.tensor_copy(kT[:, n, hp], ps_k[:])
                    for kt in range(KT):
                        ps_v = p1ps.tile([P, HC * HD], DT_F32, tag="psv")
                        for et in range(ET):
                            nc.tensor.matmul(
                                ps_v[:], xTs[:, n, et, kt * P:(kt + 1) * P],
                                wvs[:, n, et], start=(et == 0), stop=(et == ET - 1))
                        nc.vector.tensor_copy(
                            vex[:, n, kt, :, 0:HD],
                            ps_v[:].rearrange("p (h d) -> p h d", d=HD))

            # ---------------- Phase 2: cross-stream attention ----------------
            with tc.tile_pool(name="a_sps", bufs=2, space="PSUM") as a_sps, \
                 tc.tile_pool(name="a_ups", bufs=2, space="PSUM") as a_ups, \
                 tc.tile_pool(name="a_tps", bufs=2, space="PSUM") as a_tps, \
                 tc.tile_pool(name="a_sb", bufs=4) as a_sb, \
                 tc.tile_pool(name="a_sm", bufs=8) as a_sm:
                for i in range(NS):
                    for j in range(NS):
                        c_ap = cmat_sb[:, (i * NS + j):(i * NS + j + 1)]
                        for hl in range(HC):
                            hp, dh = hl // 2, (hl % 2) * HD
                            exph = []
                            for half in range(2):
                                s_ps = a_sps.tile([P, 2, S], DT_F32, tag="sps")
                                ex = a_sb.tile([P, 2, S], DT_BF, tag="exp")
                                for k2 in range(2):
                                    kt = half * 2 + k2
                                    nc.tensor.matmul(
                                        s_ps[:, k2],
                                        kT[dh:dh + HD, j, hp, kt * P:(kt + 1) * P],
                                        qT[dh:dh + HD, i, hp],
                                        start=True, stop=True)
                                nc.scalar.activation(ex[:], s_ps[:], AF.Exp,
                                                     scale=c_ap)
                                exph.append(ex)
                            ua_ps = a_ups.tile([HD + 1, S], DT_F32, tag="ua")
                            for kt in range(KT):
                                nc.tensor.matmul(
                                    ua_ps[:], vex[:, j, kt, hl],
                                    exph[kt // 2][:, kt % 2],
                                    start=(kt == 0), stop=(kt == KT - 1))
                            ua_sb = a_sb.tile([HD + 1, S], DT_BF, tag="uasb")
                            nc.vector.tensor_copy(ua_sb[:], ua_ps[:])
                            for qt in range(KT):
                                tr_ps = a_tps.tile([P, HD + 1], DT_BF, tag="tr")
                                nc.tensor.transpose(
                                    tr_ps[:], ua_sb[:, qt * P:(qt + 1) * P],
                                    ident[0:HD + 1, 0:HD + 1])
                                r_sb = a_sm.tile([P, 1], DT_F32, tag="rr")
                                nc.vector.reciprocal(r_sb[:], tr_ps[:, HD:HD + 1])
                                nc.vector.scalar_tensor_tensor(
                                    out=att[:, i, qt, hl * HD:(hl + 1) * HD],
                                    in0=tr_ps[:, 0:HD], scalar=r_sb[:],
                                    in1=att[:, i, qt, hl * HD:(hl + 1) * HD],
                                    op0=ALU.mult, op1=ALU.add)

            scopeA.release()

            # ---------------- Phase 3: Wo proj + residual + LN1 ----------------
            with tc.tile_pool(name="c_w", bufs=1) as c_w, \
                 tc.tile_pool(name="c_ps", bufs=2, space="PSUM") as c_ps, \
                 tc.tile_pool(name="c_tp", bufs=2, space="PSUM") as c_tp, \
                 tc.tile_pool(name="c_sb", bufs=3) as c_sb, \
                 tc.tile_pool(name="c_sm", bufs=6) as c_sm:
                wos = c_w.tile([P, NS, ET, E], DT_BF)
                bob = c_w.tile([P, NS, E], DT_F32)
                g1b = c_w.tile([P, NS, E], DT_F32)
                b1b = c_w.tile([P, NS, E], DT_F32)
                for n in range(NS):
                    nc.sync.dma_start(wos[:, n], wo_d[n])
                    nc.sync.dma_start(bob[:, n], bo_d[n].partition_broadcast(P))
                    nc.sync.dma_start(g1b[:, n], g1_d[n].partition_broadcast(P))
                    nc.sync.dma_start(b1b[:, n], b1_d[n].partition_broadcast(P))

                for i in range(NS):
                    nc.vector.tensor_copy(attb[:, i], att[:, i])
                    for ts in range(TS):
                        wo_ps = c_ps.tile([P, E], DT_F32, tag="wops")
                        for qt in range(KT):
                            nc.tensor.matmul(
                                wo_ps[:], attb[:, i, qt, ts * P:(ts + 1) * P],
                                wos[:, i, qt], start=(qt == 0), stop=(qt == KT - 1))
                        y1 = c_sb.tile([P, E], DT_F32, tag="y1")
                        nc.vector.tensor_add(y1[:], wo_ps[:], bob[:, i])
                        nc.vector.tensor_add(y1[:], y1[:], x32[:, i, ts])
                        # LayerNorm 1
                        nm = c_sm.tile([P, 1], DT_F32, tag="nm")
                        nc.vector.reduce_sum(nm[:], y1[:], axis=AX.X)
                        nc.vector.tensor_scalar_mul(nm[:], nm[:], -1.0 / E)
                        xc = c_sb.tile([P, E], DT_F32, tag="xc")
                        nc.vector.tensor_scalar_add(xc[:], y1[:], nm[:])
                        var = c_sm.tile([P, 1], DT_F32, tag="var")
                        sq = c_sb.tile([P, E], DT_F32, tag="sq")
                        nc.vector.scalar_tensor_tensor(
                            out=sq[:], in0=xc[:], scalar=1.0, in1=xc[:],
                            op0=ALU.mult, op1=ALU.mult, accum_out=var[:])
                        inv = c_sm.tile([P, 1], DT_F32, tag="inv")
                        nc.scalar.activation(inv[:], var[:], AF.Sqrt,
                                             bias=eps_sb[:], scale=1.0 / E)
                        nc.vector.reciprocal(inv[:], inv[:])
                        nc.vector.scalar_tensor_tensor(
                            out=r1[:, i, ts], in0=xc[:], scalar=inv[:],
                            in1=g1b[:, i], op0=ALU.mult, op1=ALU.mult)
                        nc.vector.tensor_add(r1[:, i, ts], r1[:, i, ts], b1b[:, i])
                        r1bf = c_sb.tile([P, E], DT_BF, tag="r1bf")
                        nc.vector.tensor_copy(r1bf[:], r1[:, i, ts])
                        for et in range(ET):
                            rt_ps = c_tp.tile([P, P], DT_BF, tag="rt")
                            nc.tensor.transpose(
                                rt_ps[:], r1bf[:, et * P:(et + 1) * P], ident[:])
                            nc.vector.tensor_copy(
                                r1T[:, i, et, ts * P:(ts + 1) * P], rt_ps[:])

            scopeB.release()

            # ---------------- Phase 4+5: FFN + LN2 ----------------
            with tc.tile_pool(name="f_w", bufs=2) as f_w, \
                 tc.tile_pool(name="f_c", bufs=1) as f_c, \
                 tc.tile_pool(name="f_ps", bufs=2, space="PSUM") as f_ps, \
                 tc.tile_pool(name="f_ps2", bufs=2, space="PSUM") as f_ps2, \
                 tc.tile_pool(name="f_sb", bufs=2) as f_sb, \
                 tc.tile_pool(name="f_sb2", bufs=3) as f_sb2, \
                 tc.tile_pool(name="f_sm", bufs=6) as f_sm:
                bf1s = f_c.tile([P, NS, FT], DT_F32)
                g2b = f_c.tile([P, NS, E], DT_F32)
                b2b = f_c.tile([P, NS, E], DT_F32)
                bf2b = f_c.tile([P, NS, E], DT_F32)
                for n in range(NS):
                    nc.sync.dma_start(bf1s[:, n], bf1_d[n])
                    nc.sync.dma_start(g2b[:, n], g2_d[n].partition_broadcast(P))
                    nc.sync.dma_start(b2b[:, n], b2_d[n].partition_broadcast(P))
                    nc.sync.dma_start(bf2b[:, n], bf2_d[n].partition_broadcast(P))

                for n in range(NS):
                    w1s = f_w.tile([P, ET, 4 * E], DT_BF, tag="w1s")
                    w2s = f_w.tile([P, FT, E], DT_BF, tag="w2s")
                    nc.sync.dma_start(w1s[:], w1_d[n])
                    nc.sync.dma_start(w2s[:], w2_d[n])
                    hT = f_sb.tile([P, FT, TG], DT_BF, tag="hT")
                    for fs in range(FT):
                        h_ps = f_ps.tile([P, TG], DT_F32, tag="hps")
                        for et in range(ET):
                            nc.tensor.matmul(
                                h_ps[:], w1s[:, et, fs * P:(fs + 1) * P],
                                r1T[:, n, et], start=(et == 0), stop=(et == ET - 1))
                        nc.scalar.activation(hT[:, fs], h_ps[:], AF.Gelu,
                                             bias=bf1s[:, n, fs:fs + 1])
                    out_sb = f_sb.tile([P, TS, E], DT_F32, tag="outsb")
                    for ts in range(TS):
                        f2_ps = f_ps2.tile([P, E], DT_F32, tag="fps")
                        for ft in range(FT):
                            nc.tensor.matmul(
                                f2_ps[:], hT[:, ft, ts * P:(ts + 1) * P],
                                w2s[:, ft], start=(ft == 0), stop=(ft == FT - 1))
                        y2 = f_sb2.tile([P, E], DT_F32, tag="y2")
                        nc.vector.tensor_add(y2[:], f2_ps[:], bf2b[:, n])
                        nc.vector.tensor_add(y2[:], y2[:], r1[:, n, ts])
                        # LayerNorm 2
                        nm = f_sm.tile([P, 1], DT_F32, tag="nm2")
                        nc.vector.reduce_sum(nm[:], y2[:], axis=AX.X)
                        nc.vector.tensor_scalar_mul(nm[:], nm[:], -1.0 / E)
                        xc = f_sb2.tile([P, E], DT_F32, tag="xc2")
                        nc.vector.tensor_scalar_add(xc[:], y2[:], nm[:])
                        var = f_sm.tile([P, 1], DT_F32, tag="var2")
                        sq = f_sb2.tile([P, E], DT_F32, tag="sq2")
                        nc.vector.scalar_tensor_tensor(
                            out=sq[:], in0=xc[:], scalar=1.0, in1=xc[:],
                            op0=ALU.mult, op1=ALU.mult, accum_out=var[:])
                        inv = f_sm.tile([P, 1], DT_F32, tag="inv2")
                        nc.scalar.activation(inv[:], var[:], AF.Sqrt,
                                             bias=eps_sb[:], scale=1.0 / E)
                        nc.vector.reciprocal(inv[:], inv[:])
                        nc.vector.scalar_tensor_tensor(
                            out=out_sb[:, ts], in0=xc[:], scalar=inv[:],
                            in1=g2b[:, n], op0=ALU.mult, op1=ALU.mult)
                        nc.vector.tensor_add(out_sb[:, ts], out_sb[:, ts],
                                             b2b[:, n])
                    nc.sync.dma_start(out_d[n], out_sb[:])

    nc.compile()
    return nc


_NC_CACHE = {}


def _get_nc():
    if "nc" not in _NC_CACHE:
        _NC_CACHE["nc"] = _build_program()
    return _NC_CACHE["nc"]


def _pack_inputs(x0, x1, x2, x3, Wq, Wk, Wv, Wo, bo, ln1_g, ln1_b, ln2_g, ln2_b,
                 W1, bf1, W2, bf2, inter):
    x = np.stack([np.asarray(x0), np.asarray(x1), np.asarray(x2),
                  np.asarray(x3)]).astype(F32)  # [NS,B,S,E]
    Wq, Wk, Wv, Wo = (np.asarray(a, F32) for a in (Wq, Wk, Wv, Wo))
    W1, W2 = np.asarray(W1, F32), np.asarray(W2, F32)
    inter = np.asarray(inter, F32)

    def tile_rows(a, nt):
        # [NS, R, C] -> [NS, P, nt, C]
        return np.ascontiguousarray(
            a.reshape(NS, nt, P, a.shape[-1]).transpose(0, 2, 1, 3))

    shared = {
        "wo": tile_rows(Wo / NS, ET).astype(BF16),
        "w1": tile_rows(W1, ET).astype(BF16),
        "w2": tile_rows(W2, FT).astype(BF16),
        "cmat": np.ascontiguousarray(
            np.broadcast_to((inter * SCALE).reshape(1, NS * NS), (P, NS * NS))
        ).astype(F32),
        "bo": np.ascontiguousarray(bo, dtype=F32),
        "g1": np.ascontiguousarray(ln1_g, dtype=F32),
        "b1": np.ascontiguousarray(ln1_b, dtype=F32),
        "g2": np.ascontiguousarray(ln2_g, dtype=F32),
        "b2": np.ascontiguousarray(ln2_b, dtype=F32),
        "bf1": np.ascontiguousarray(
            np.asarray(bf1, F32).reshape(NS, FT, P).transpose(0, 2, 1)),
        "bf2": np.ascontiguousarray(bf2, dtype=F32),
    }
    per_hg = []
    for hg in range(HG):
        cols = slice(hg * HC * HD, (hg + 1) * HC * HD)
        per_hg.append({
            "wq": tile_rows(Wq[:, :, cols], ET).astype(BF16),
            "wk": tile_rows(Wk[:, :, cols], ET).astype(BF16),
            "wv": tile_rows(Wv[:, :, cols], ET).astype(BF16),
        })
    in_maps = []
    for core in range(N_CORES):
        b, hg = core // HG, core % HG
        xb = x[:, b]  # [NS, S, E]
        xT = np.ascontiguousarray(
            xb.transpose(0, 2, 1).reshape(NS, ET, P, S).transpose(0, 2, 1, 3)
        ).astype(BF16)
        x32 = np.ascontiguousarray(
            xb[:, hg * TG:(hg + 1) * TG].reshape(NS, TS, P, E)
            .transpose(0, 2, 1, 3))
        m = {"xT": xT, "x32": x32}
        m.update(shared)
        m.update(per_hg[hg])
        in_maps.append(m)
    return in_maps


def _unpack_outputs(results):
    full = np.empty((NS, B, S, E), dtype=F32)
    for core in range(N_CORES):
        b, hg = core // HG, core % HG
        o = results[core]["out"]  # [NS, P, TS, E]
        full[:, b, hg * TG:(hg + 1) * TG] = (
            o.transpose(0, 2, 1, 3).reshape(NS, TG, E))
    return tuple(full[n] for n in range(NS))


def kernel(**inputs):
    nc = _get_nc()
    in_maps = _pack_inputs(**inputs)
    res = run_bass_kernel_spmd(
        nc, in_maps, core_ids=list(range(N_CORES)),
        trace=bool(int(os.environ.get("KERNEL_TRACE", "0"))))
    _NC_CACHE["last_result"] = res
    return _unpack_outputs(res.results)


def bench(inputs, iters=20):
    """Time the on-device execution with device-resident inputs.

    Mirrors bass2jax.run_bass_via_pjrt's shard_map(_bass_exec) lowering but
    without output-buffer donation, so the same executable can be re-invoked
    and timed. Returns (min, median) seconds per call.
    """
    import time
    import jax
    import jax.numpy as jnp
    from jax.sharding import Mesh, PartitionSpec, NamedSharding
    from jax.experimental.shard_map import shard_map
    from concourse import bass2jax
    from concourse import mybir as mb

    nc = _get_nc()
    bass2jax.install_neuronx_cc_hook()
    in_maps = _pack_inputs(**inputs)

    part_name = nc.partition_id_tensor.name if nc.partition_id_tensor else None
    in_names, out_names, out_avals, zero_outs = [], [], [], []
    for alloc in nc.m.functions[0].allocations:
        if not isinstance(alloc, mb.MemoryLocationSet):
            continue
        name = alloc.memorylocations[0].name
        if alloc.kind == "ExternalInput":
            if name != part_name:
                in_names.append(name)
        elif alloc.kind == "ExternalOutput":
            out_names.append(name)
            shape = tuple(alloc.tensor_shape)
            dtype = mb.dt.np(alloc.dtype)
            out_avals.append(jax.core.ShapedArray(shape, dtype))
            zero_outs.append(np.zeros(shape, dtype))
    n_params = len(in_names)
    all_names = in_names + out_names
    if part_name is not None:
        all_names = all_names + [part_name]

    def _body(*args):
        operands = list(args)
        if part_name is not None:
            operands.append(bass2jax.partition_id_tensor())
        outs = bass2jax._bass_exec_p.bind(
            *operands, out_avals=tuple(out_avals), in_names=tuple(all_names),
            out_names=tuple(out_names), lowering_input_output_aliases=(),
            sim_require_finite=True, sim_require_nnan=True, nc=nc)
        return tuple(outs)

    devices = jax.devices()[:N_CORES]
    mesh = Mesh(np.asarray(devices), ("core",))
    spec = PartitionSpec("core")
    fn = jax.jit(shard_map(
        _body, mesh=mesh, in_specs=(spec,) * (n_params + len(out_names)),
        out_specs=(spec,) * len(out_names), check_rep=False))
    sh = NamedSharding(mesh, spec)
    concat = [jax.device_put(
        np.concatenate([in_maps[c][nm] for c in range(N_CORES)], axis=0), sh)
        for nm in in_names]
    concat += [jax.device_put(
        np.zeros((N_CORES * z.shape[0], *z.shape[1:]), z.dtype), sh)
        for z in zero_outs]

    out = fn(*concat)  # compile
    jax.block_until_ready(out)
    times = []
    for _ in range(iters):
        t0 = time.perf_counter()
        out = fn(*concat)
        jax.block_until_ready(out)
        times.append(time.perf_counter() - t0)
    times.sort()
    return times[0], times[len(times) // 2]


if __name__ == "__main__":
    import sys
    mode = sys.argv[1] if len(sys.argv) > 1 else "sim"
    sys.path.insert(0, os.path.dirname(os.path.abspath(__file__)))
    import reference

    inputs = {k: np.asarray(v) for k, v in reference.setup_inputs().items()}
    if mode == "sim":
        # Simulate core 0 (b=0, hg=0) with CoreSim and compare to reference.
        # CoreSim has no Gelu; patch exact erf-gelu into its activation visitor.
        import concourse.bass_interp as bass_interp
        from scipy.special import erf as _erf
        _orig_visit = bass_interp.InstructionExecutor.visit_InstActivation

        def _patched(self, instruction, reg_snapshot=None):
            if instruction.func == mybir.ActivationFunctionType.Gelu:
                instruction.func = mybir.ActivationFunctionType.Identity
                try:
                    import concourse.mybir as mb
                    from concourse.bass_interp import Direction
                    out_ap = instruction.outs[0]
                    res = _orig_visit(self, instruction, reg_snapshot=reg_snapshot)
                    v = self.view_ap(out_ap, Direction.WRITE, instruction,
                                     reg_snapshot=reg_snapshot)
                    x = v[:].astype(np.float32)
                    v[:] = (x * 0.5 * (1.0 + _erf(x / np.sqrt(2.0)))).astype(v.dtype)
                    return res
                finally:
                    instruction.func = mybir.ActivationFunctionType.Gelu
            return _orig_visit(self, instruction, reg_snapshot=reg_snapshot)

        bass_interp.InstructionExecutor.visit_InstActivation = _patched
        from concourse.bass_interp import CoreSim
        nc = _get_nc()
        in_maps = _pack_inputs(**inputs)
        sim = CoreSim(nc, trace=False)
        for name, arr in in_maps[0].items():
            sim.tensor(name)[:] = arr
        sim.simulate(check_with_hw=False)
        out = sim.tensor("out").copy()
        got = out.transpose(0, 2, 1, 3).reshape(NS, TG, E)
        exp = np.stack([np.asarray(o) for o in reference.reference(**inputs)])
        exp_slice = exp[:, 0, 0:TG]  # b=0, rows 0:256
        err = np.abs(got - exp_slice)
        rel = np.linalg.norm(got - exp_slice) / np.linalg.norm(exp_slice)
        print(f"max abs err: {err.max():.3e}  rel fro err: {rel:.3e}")
    else:
        got = kernel(**inputs)
        exp = reference.reference(**inputs)
        for n in range(NS):
            g, e = np.asarray(got[n]), np.asarray(exp[n])
            rel = np.linalg.norm(g - e) / np.linalg.norm(e)
            print(f"out{n}: rel fro err {rel:.3e} max abs {np.abs(g - e).max():.3e}")
